# revision 5
# baseline (speedup 1.0000x reference)
"""Chamfer loss kernel for Trainium2 — single-orientation, tt-tree version.

Single matmul orientation (pred stationary): full dist^2 = x2 + y2 -
2 x.y lands in PSUM via 13 bf16 hi/lo contraction rows.  The Act engine
evacuates each [128, 2048] psum group to SBUF bf16.  The DVE then works
entirely in bf16 (2x perf mode):
  - dist1 rows: per pred tile, a pairwise tensor_tensor-min tree over
    the 4 evacuated groups (e0&e1, e2&e3, then the pair) + one bf16
    tensor_reduce -> part1[:, t].
  - dist2 cols: tensor_tensor-min of each group into the [128, 8192]
    bf16 column accumulator; the final 128-way partition min + sqrt +
    mean run on the host.
"""

import numpy as np

_NPTS = 8192
_P = 128
_NH = _NPTS // 2
_K = 13
_T1 = _NH // _P    # 32 pred tiles
_G1 = 4            # 2048-wide groups per pred tile


def _build_kernel(repeats=1, mode="full"):
    import concourse.bacc as bacc
    import concourse.bass as bass
    import concourse.mybir as mybir
    import concourse.tile as tile

    f32 = mybir.dt.float32
    bf16 = mybir.dt.bfloat16
    mn = mybir.AluOpType.min
    X = mybir.AxisListType.X

    nc = bacc.Bacc("TRN2", target_bir_lowering=False, debug=False, num_devices=8)
    predw_d = nc.dram_tensor("predw", [_K, _NH], bf16, kind="ExternalInput")
    targx_d = nc.dram_tensor("targx", [_K, _NPTS], bf16, kind="ExternalInput")
    part1_d = nc.dram_tensor("part1", [_P, _T1], f32, kind="ExternalOutput")
    cmk_d = nc.dram_tensor("cmk", [_P, _NPTS], bf16, kind="ExternalOutput")

    do_mm = mode in ("full", "mm_only", "evac_only", "row_only", "col_only")
    do_evac = mode in ("full", "evac_only", "row_only", "col_only")
    do_row = mode in ("full", "row_only")
    do_col = mode in ("full", "col_only")

    with tile.TileContext(nc) as tc:
        with tc.tile_pool(name="const", bufs=1) as cp, \
             tc.tile_pool(name="ps", bufs=1, space="PSUM") as pp:
            predw = cp.tile([_K, _NH], bf16)
            targx = cp.tile([_K, _NPTS], bf16)
            part1 = cp.tile([_P, _T1], f32)
            acc = cp.tile([_P, _NPTS], bf16)
            rfa = cp.tile([_P, 2048], bf16)
            rfb = cp.tile([_P, 2048], bf16)
            eb = [cp.tile([_P, 2048], bf16, name=f"eb{i}") for i in range(8)]
            p0 = pp.tile([_P, 2048], f32)
            p1 = pp.tile([_P, 2048], f32)

            nc.sync.dma_start(predw[:], predw_d[:])
            nc.sync.dma_start(targx[:], targx_d[:])
            if mode != "full":
                nc.vector.memset(p0[:], 0.0)
                nc.vector.memset(p1[:], 0.0)
                nc.vector.memset(part1[:], 0.0)
                nc.vector.memset(acc[:], 0.0)
                nc.vector.memset(rfa[:], 0.0)
                nc.vector.memset(rfb[:], 0.0)
                for e_ in eb:
                    nc.vector.memset(e_[:], 0.0)

            # Unroll 2 passes per hardware-loop iteration: halves the
            # all-engine barrier count and lets pass k+1's matmuls
            # overlap pass k's vector tail (passes are idempotent:
            # min-folds of identical data converge to the same result).
            unroll = 4 if repeats % 4 == 0 else (2 if repeats % 2 == 0 else 1)
            with tc.For_i(0, repeats // unroll) as _:
              for _u in range(unroll):
                gi = 0
                for t in range(_T1):
                    w = predw[:, t * _P:(t + 1) * _P]
                    es = []
                    for g in range(_G1):
                        pt = p0 if gi % 2 == 0 else p1
                        e = eb[gi % 8]
                        es.append(e)
                        if do_mm:
                            for k in range(4):
                                c = (g * 4 + k) * 512
                                nc.tensor.matmul(
                                    pt[:, k * 512:(k + 1) * 512],
                                    w, targx[:, c:c + 512],
                                )
                        if do_evac:
                            nc.scalar.copy(e[:], pt[:])
                        if do_row and g == 1:
                            nc.vector.tensor_tensor(rfa[:], es[0][:], es[1][:], op=mn)
                        if do_row and g == 3:
                            nc.vector.tensor_tensor(rfb[:], es[2][:], es[3][:], op=mn)
                            nc.vector.tensor_tensor(rfa[:], rfa[:], rfb[:], op=mn)
                            nc.vector.tensor_reduce(
                                part1[:, t:t + 1], rfa[:], axis=X, op=mn)
                        if do_col:
                            a = acc[:, g * 2048:(g + 1) * 2048]
                            if t == 0:
                                nc.vector.tensor_copy(a, e[:])
                            else:
                                nc.vector.tensor_tensor(a, e[:], a, op=mn)
                        gi += 1

            nc.sync.dma_start(part1_d[:], part1[:])
            nc.sync.dma_start(cmk_d[:], acc[:])

    nc.compile()
    return nc


_NC_CACHE = None


def _get_nc():
    global _NC_CACHE
    if _NC_CACHE is None:
        _NC_CACHE = _build_kernel()
    return _NC_CACHE


def _hilo(x):
    import ml_dtypes
    hi = x.astype(ml_dtypes.bfloat16)
    lo = (x - hi.astype(np.float32)).astype(ml_dtypes.bfloat16)
    return hi, lo


def _prep_core(predh, target):
    import ml_dtypes
    bf = ml_dtypes.bfloat16
    a, b = _hilo(predh.astype(np.float32))
    c, e = _hilo(target.astype(np.float32))
    x2 = np.sum(predh.astype(np.float64) ** 2, axis=-1).astype(np.float32)
    y2 = np.sum(target.astype(np.float64) ** 2, axis=-1).astype(np.float32)
    x2h, x2l = _hilo(x2)
    y2h, y2l = _hilo(y2)

    af = a.astype(np.float32)
    bf32 = b.astype(np.float32)
    cf = c.astype(np.float32)
    ef = e.astype(np.float32)
    ones_n = np.ones(_NH, dtype=bf)
    ones_m = np.ones(_NPTS, dtype=bf)

    def rows(lst):
        return np.ascontiguousarray(np.stack([np.asarray(r, dtype=bf) for r in lst]))

    predw = rows([(-2 * af[:, 0]), (-2 * af[:, 1]), (-2 * af[:, 2]),
                  (-2 * af[:, 0]), (-2 * af[:, 1]), (-2 * af[:, 2]),
                  (-2 * bf32[:, 0]), (-2 * bf32[:, 1]), (-2 * bf32[:, 2]),
                  x2h, x2l, ones_n, ones_n])
    targx = rows([cf[:, 0], cf[:, 1], cf[:, 2],
                  ef[:, 0], ef[:, 1], ef[:, 2],
                  cf[:, 0], cf[:, 1], cf[:, 2],
                  ones_m, ones_m, y2h, y2l])
    return {"predw": predw, "targx": targx}


def _prep_in_maps(pred, target):
    pred = np.asarray(pred, dtype=np.float32)
    target = np.asarray(target, dtype=np.float32)
    B = pred.shape[0]
    in_maps = []
    for bidx in range(B):
        for h in range(2):
            in_maps.append(_prep_core(pred[bidx, h * _NH:(h + 1) * _NH], target[bidx]))
    return in_maps


_LAST_RESULT = None


def kernel(pred, target):
    from concourse.bass_utils import run_bass_kernel_spmd

    B = np.asarray(pred).shape[0]
    in_maps = _prep_in_maps(pred, target)
    nc = _get_nc()
    res = run_bass_kernel_spmd(nc, in_maps, list(range(2 * B)))
    global _LAST_RESULT
    _LAST_RESULT = res

    total = 0.0
    for bidx in range(B):
        ra, rb = res.results[2 * bidx], res.results[2 * bidx + 1]
        d1a = np.sqrt(np.maximum(
            ra["part1"].astype(np.float64).T.reshape(_NH), 0.0))
        d1b = np.sqrt(np.maximum(
            rb["part1"].astype(np.float64).T.reshape(_NH), 0.0))
        ch1 = 0.5 * (d1a.mean() + d1b.mean())
        m2 = np.minimum(ra["cmk"].astype(np.float32),
                        rb["cmk"].astype(np.float32)).min(axis=0)
        ch2 = np.sqrt(np.maximum(m2.astype(np.float64), 0.0)).mean()
        total += ch1 + ch2
    return np.float32(total / B)
